# revision 4
# baseline (speedup 1.0000x reference)
"""BERT embedding lookup on 8 TRN2 NeuronCores — bf16 + PE-base version.

Strategy: sequence-parallel — core c handles positions [64c, 64c+64) of
all 32 batch rows (2048 tokens, 16 tiles of 128; tile t = batch rows
(2t, 2t+1), partition p = (batch parity p//64, s_local p%64)).

Word table converted to bf16 on the host (rel err ~2^-9, inside the 2e-2
gate): halves gather HBM bytes. Gathers are 16 generic SWDGE indirect
DMAs of 128 rows each (the HW ucode supports exactly one offset per
partition per op), issued back-to-back with no pool-recycling stalls so
the ~1.4us/op Q7 descriptor-generation pace is the only serialization.

The position+type base (posA[s] + tt*diff) is built on the otherwise
IDLE PE as K=66 matmuls per PSUM bank: lhsT rows 0-63 are a onehot(p%64)
pattern selecting posA rows, rows 64-65 are the chunk's two tt columns
selecting diff. DVE then does a single pass per chunk:
res_f32 = wt*1 + psum_base (stt) — a second DVE pass would cost ~15us
extra against a ~30us roofline.

Compute/store chunks taper at both ends (1,1,2,2,2,2,2,2,1,1 tiles)
so stores start early (filling the SDMA engines during the gather ramp)
and the tail after the last gather is short; stores alternate
sync/scalar HWDGE queues.
Host reassembles [32, 512, 768] from the [128, 16, 768] per-core outs.
"""

import numpy as np

P = 128
H = 768
VOCAB = 30522
SEQ = 512
BATCH = 32
N_CORES = 8
S_PER_CORE = SEQ // N_CORES  # 64
TOK_PER_CORE = BATCH * S_PER_CORE  # 2048
T_TILES = TOK_PER_CORE // P  # 16
CHUNK_TILES = [1, 1, 1, 2, 2, 2, 2, 2, 2, 1]  # per compute/store chunk
C_CHUNKS = len(CHUNK_TILES)
CHUNK_T0 = [sum(CHUNK_TILES[:i]) for i in range(C_CHUNKS)]
K_LHS = 66  # 64 onehot rows + 2 tt rows

_CACHE = {}


def _build():
    from concourse import bacc, mybir
    import concourse.bass as bass
    import concourse.tile as tile

    nc = bacc.Bacc(
        "TRN2",
        target_bir_lowering=False,
        debug=False,
        num_devices=N_CORES,
        dynamic_dma_scratch_size=131072,
        num_swdge_queues=1,
    )
    f32 = mybir.dt.float32
    bf16 = mybir.dt.bfloat16
    i32 = mybir.dt.int32

    wemb = nc.dram_tensor("wemb", [VOCAB, H], bf16, kind="ExternalInput").ap()
    # fused matmul consts: lhs [66, C*128] | rhs [66, 1536]
    mmb = nc.dram_tensor(
        "mmb", [K_LHS, C_CHUNKS * P + 2 * H], bf16, kind="ExternalInput"
    ).ap()
    ids = nc.dram_tensor("ids", [P, T_TILES], i32, kind="ExternalInput").ap()
    out = nc.dram_tensor("out", [P, T_TILES, H], f32, kind="ExternalOutput").ap()

    with tile.TileContext(nc) as tc:
        with (
            tc.tile_pool(name="consts", bufs=1) as consts,
            tc.tile_pool(name="resp", bufs=5) as rpool,
            tc.tile_pool(name="psum", bufs=2, space="PSUM") as ppool,
        ):
            ids_sb = consts.tile([P, T_TILES], i32)
            # scalar HWDGE issues earliest after the prologue; ids first so
            # the gather stream unblocks as soon as possible
            nc.scalar.dma_start(out=ids_sb[:], in_=ids[:])
            mm_sb = consts.tile([K_LHS, C_CHUNKS * P + 2 * H], bf16)
            nc.sync.dma_start(out=mm_sb[:], in_=mmb[:])
            lhs_sb = mm_sb[:, : C_CHUNKS * P]
            rhs_sb = mm_sb[:, C_CHUNKS * P :]

            # all gather destinations live for the whole kernel: the 16
            # gathers issue back-to-back on Q7 with no pool-recycling stalls
            wts = [
                consts.tile([P, CHUNK_TILES[g] * H], bf16, name=f"wt{g}")
                for g in range(C_CHUNKS)
            ]
            for g in range(C_CHUNKS):
                for k in range(CHUNK_TILES[g]):
                    t = CHUNK_T0[g] + k
                    # HW ucode semantics: ONE offset per partition per op
                    nc.gpsimd.indirect_dma_start(
                        out=wts[g][:, k * H : (k + 1) * H],
                        out_offset=None,
                        in_=wemb[:],
                        in_offset=bass.IndirectOffsetOnAxis(
                            ap=ids_sb[:, t : t + 1], axis=0
                        ),
                    )
            for g in range(C_CHUNKS):
                cw = CHUNK_TILES[g] * H
                base_full = ppool.tile([P, 2 * H], f32, name="base")
                base = base_full[:, :cw]
                for b0 in range(0, cw, 512):
                    b1 = min(b0 + 512, cw)
                    # base[p, n] = posA[p, n] + tt0[p]*diffA[n] + tt1[p]*diffB[n]
                    nc.tensor.matmul(
                        base_full[:, b0:b1],
                        lhs_sb[:, g * P : (g + 1) * P],
                        rhs_sb[:, b0:b1],
                        start=True,
                        stop=True,
                    )
                res_full = rpool.tile([P, 2 * H], f32, name="res")
                # res = wt * 1 + base  (single DVE pass, f32 out)
                nc.vector.scalar_tensor_tensor(
                    out=res_full[:, :cw],
                    in0=wts[g][:],
                    scalar=1.0,
                    in1=base,
                    op0=mybir.AluOpType.mult,
                    op1=mybir.AluOpType.add,
                )
                eng = nc.sync if g % 2 == 0 else nc.scalar
                t0 = CHUNK_T0[g]
                eng.dma_start(
                    out=out[:, t0 : t0 + CHUNK_TILES[g], :],
                    in_=res_full[:, :cw],
                )

    nc.compile()
    return nc


def _get_nc():
    if "nc" not in _CACHE:
        _CACHE["nc"] = _build()
    return _CACHE["nc"]


def _prep_inputs(
    input_ids, token_type_ids, word_embedding, position_embedding, token_type_embedding
):
    import ml_dtypes

    bf16 = ml_dtypes.bfloat16
    wemb = np.ascontiguousarray(np.asarray(word_embedding, dtype=np.float32)).astype(
        bf16
    )
    pos = np.asarray(position_embedding, dtype=np.float32)
    typ = np.asarray(token_type_embedding, dtype=np.float32)
    diff = typ[1] - typ[0]  # [H]
    posA = pos + typ[0][None, :]  # [SEQ, H]

    # token (core c, b, s_local) -> tile t = b//2, partition p = (b%2)*64 + s_local
    ids_np = np.asarray(input_ids, dtype=np.int32).reshape(
        BATCH, N_CORES, S_PER_CORE
    )  # [b, c, s]
    tt_np = np.asarray(token_type_ids, dtype=np.float32).reshape(
        BATCH, N_CORES, S_PER_CORE
    )

    def to_tp(x):  # [b, c, s] -> [c, t, p]
        x = x.reshape(T_TILES, 2, N_CORES, S_PER_CORE)  # [t, par, c, s]
        return x.transpose(2, 0, 1, 3).reshape(N_CORES, T_TILES, P)

    ids_tp = to_tp(ids_np)  # [c, 16, 128]
    tt_tp = to_tp(tt_np)

    ids_dram = np.ascontiguousarray(ids_tp.transpose(0, 2, 1))  # [c, p, t] int32

    # lhsb [66, C*128]: rows 0-63 onehot(p%64); rows 64/65 = tt of the
    # chunk's first/second tile (row 65 zero for 1-tile chunks: its rhs
    # half is outside the chunk's slice anyway)
    onehot = np.zeros((64, P), dtype=np.float32)
    onehot[np.arange(P) % 64, np.arange(P)] = 1.0
    lhsb = np.zeros((N_CORES, K_LHS, C_CHUNKS * P), dtype=bf16)
    for g in range(C_CHUNKS):
        lhsb[:, :64, g * P : (g + 1) * P] = onehot.astype(bf16)[None]
        lhsb[:, 64, g * P : (g + 1) * P] = tt_tp[:, CHUNK_T0[g], :].astype(bf16)
        if CHUNK_TILES[g] > 1:
            lhsb[:, 65, g * P : (g + 1) * P] = tt_tp[:, CHUNK_T0[g] + 1, :].astype(
                bf16
            )

    # rhsb [66, 1536]: rows 0-63 = [posA64 | posA64]; row 64 = [diff|0]; row 65 = [0|diff]
    rhsb = np.zeros((N_CORES, K_LHS, 2 * H), dtype=bf16)
    for c in range(N_CORES):
        pa = posA[c * S_PER_CORE : (c + 1) * S_PER_CORE].astype(bf16)  # [64, H]
        pa2 = np.concatenate([pa, pa], axis=0)  # [128, H]
        rhsb[c, :64, :H] = pa
        rhsb[c, :64, H:] = pa
    rhsb[:, 64, :H] = diff.astype(bf16)[None]
    rhsb[:, 65, H:] = diff.astype(bf16)[None]

    mmb = np.concatenate([lhsb, rhsb], axis=2)  # [c, 66, C*128 + 2H]
    return [
        {
            "wemb": wemb,
            "mmb": np.ascontiguousarray(mmb[c]),
            "ids": np.ascontiguousarray(ids_dram[c]),
        }
        for c in range(N_CORES)
    ]


def kernel(
    input_ids, token_type_ids, word_embedding, position_embedding, token_type_embedding
):
    from concourse.bass_utils import run_bass_kernel_spmd

    nc = _get_nc()
    in_maps = _prep_inputs(
        input_ids,
        token_type_ids,
        word_embedding,
        position_embedding,
        token_type_embedding,
    )
    r = run_bass_kernel_spmd(nc, in_maps, core_ids=list(range(N_CORES)))
    # per-core out [128, 16, 768]: token (t*128+p) = (b=2t+p//64, s=p%64)
    cores = []
    for c in range(N_CORES):
        o = np.asarray(r.results[c]["out"])  # [128, 16, 768]
        o = o.reshape(2, S_PER_CORE, T_TILES, H)  # [par, s, t, h]
        o = o.transpose(2, 0, 1, 3).reshape(BATCH, S_PER_CORE, H)  # [b, s, h]
        cores.append(o)
    full = np.stack(cores, axis=0)  # [c, b, s, h]
    full = full.transpose(1, 0, 2, 3).reshape(BATCH, SEQ, H)
    return np.ascontiguousarray(full)


# revision 5
# speedup vs baseline: 1.1294x; 1.1294x over previous
"""BERT embedding lookup on 8 TRN2 NeuronCores — bf16 + PE-base version.

Strategy: sequence-parallel — core c handles positions [64c, 64c+64) of
all 32 batch rows (2048 tokens, 16 tiles of 128; tile t = batch rows
(2t, 2t+1), partition p = (batch parity p//64, s_local p%64)).

Word table converted to bf16 on the host (rel err ~2^-9, inside the 2e-2
gate): halves gather HBM bytes. Gathers are 16 generic SWDGE indirect
DMAs of 128 rows each (the HW ucode supports exactly one offset per
partition per op), issued back-to-back with no pool-recycling stalls so
the ~1.4us/op Q7 descriptor-generation pace is the only serialization.

The position+type base (posA[s] + tt*diff) is built on the otherwise
IDLE PE as K=66 matmuls per PSUM bank: lhsT rows 0-63 are a onehot(p%64)
pattern selecting posA rows, rows 64-65 are the chunk's two tt columns
selecting diff. DVE then does a single pass per chunk:
res_f32 = wt*1 + psum_base (stt) — a second DVE pass would cost ~15us
extra against a ~30us roofline.

Compute/store chunks taper at both ends (1,1,2,2,2,2,2,2,1,1 tiles)
so stores start early (filling the SDMA engines during the gather ramp)
and the tail after the last gather is short; stores alternate
sync/scalar HWDGE queues.
Host reassembles [32, 512, 768] from the [128, 16, 768] per-core outs.
"""

import numpy as np

P = 128
H = 768
VOCAB = 30522
SEQ = 512
BATCH = 32
N_CORES = 8
S_PER_CORE = SEQ // N_CORES  # 64
TOK_PER_CORE = BATCH * S_PER_CORE  # 2048
T_TILES = TOK_PER_CORE // P  # 16
CHUNK_TILES = [1, 1, 2, 2, 2, 2, 2, 2, 1, 1]  # per compute/store chunk
C_CHUNKS = len(CHUNK_TILES)
CHUNK_T0 = [sum(CHUNK_TILES[:i]) for i in range(C_CHUNKS)]
K_LHS = 66  # 64 onehot rows + 2 tt rows

_CACHE = {}


def _build():
    from concourse import bacc, mybir
    import concourse.bass as bass
    import concourse.tile as tile

    nc = bacc.Bacc(
        "TRN2",
        target_bir_lowering=False,
        debug=False,
        num_devices=N_CORES,
        dynamic_dma_scratch_size=131072,
        num_swdge_queues=1,
    )
    f32 = mybir.dt.float32
    bf16 = mybir.dt.bfloat16
    i32 = mybir.dt.int32

    wemb = nc.dram_tensor("wemb", [VOCAB, H], bf16, kind="ExternalInput").ap()
    # fused matmul consts: lhs [66, C*128] | rhs [66, 1536]
    mmb = nc.dram_tensor(
        "mmb", [K_LHS, C_CHUNKS * P + 2 * H], bf16, kind="ExternalInput"
    ).ap()
    ids = nc.dram_tensor("ids", [P, T_TILES], i32, kind="ExternalInput").ap()
    out = nc.dram_tensor("out", [P, T_TILES, H], f32, kind="ExternalOutput").ap()

    with tile.TileContext(nc) as tc:
        with (
            tc.tile_pool(name="consts", bufs=1) as consts,
            tc.tile_pool(name="resp", bufs=5) as rpool,
            tc.tile_pool(name="psum", bufs=2, space="PSUM") as ppool,
        ):
            ids_sb = consts.tile([P, T_TILES], i32)
            # scalar HWDGE issues earliest after the prologue; ids first so
            # the gather stream unblocks as soon as possible
            nc.scalar.dma_start(out=ids_sb[:], in_=ids[:])
            mm_sb = consts.tile([K_LHS, C_CHUNKS * P + 2 * H], bf16)
            nc.scalar.dma_start(out=mm_sb[:], in_=mmb[:])
            lhs_sb = mm_sb[:, : C_CHUNKS * P]
            rhs_sb = mm_sb[:, C_CHUNKS * P :]

            # all gather destinations live for the whole kernel: the 16
            # gathers issue back-to-back on Q7 with no pool-recycling stalls
            wts = [
                consts.tile([P, CHUNK_TILES[g] * H], bf16, name=f"wt{g}")
                for g in range(C_CHUNKS)
            ]
            for g in range(C_CHUNKS):
                for k in range(CHUNK_TILES[g]):
                    t = CHUNK_T0[g] + k
                    # HW ucode semantics: ONE offset per partition per op
                    nc.gpsimd.indirect_dma_start(
                        out=wts[g][:, k * H : (k + 1) * H],
                        out_offset=None,
                        in_=wemb[:],
                        in_offset=bass.IndirectOffsetOnAxis(
                            ap=ids_sb[:, t : t + 1], axis=0
                        ),
                    )
            for g in range(C_CHUNKS):
                cw = CHUNK_TILES[g] * H
                base_full = ppool.tile([P, 2 * H], f32, name="base")
                base = base_full[:, :cw]
                for b0 in range(0, cw, 512):
                    b1 = min(b0 + 512, cw)
                    # base[p, n] = posA[p, n] + tt0[p]*diffA[n] + tt1[p]*diffB[n]
                    nc.tensor.matmul(
                        base_full[:, b0:b1],
                        lhs_sb[:, g * P : (g + 1) * P],
                        rhs_sb[:, b0:b1],
                        start=True,
                        stop=True,
                    )
                res_full = rpool.tile([P, 2 * H], f32, name="res")
                # res = wt * 1 + base  (single DVE pass, f32 out)
                nc.vector.scalar_tensor_tensor(
                    out=res_full[:, :cw],
                    in0=wts[g][:],
                    scalar=1.0,
                    in1=base,
                    op0=mybir.AluOpType.mult,
                    op1=mybir.AluOpType.add,
                )
                eng = nc.sync if g % 2 == 0 else nc.scalar
                t0 = CHUNK_T0[g]
                eng.dma_start(
                    out=out[:, t0 : t0 + CHUNK_TILES[g], :],
                    in_=res_full[:, :cw],
                )

    nc.compile()
    return nc


def _get_nc():
    if "nc" not in _CACHE:
        _CACHE["nc"] = _build()
    return _CACHE["nc"]


def _prep_inputs(
    input_ids, token_type_ids, word_embedding, position_embedding, token_type_embedding
):
    import ml_dtypes

    bf16 = ml_dtypes.bfloat16
    wemb = np.ascontiguousarray(np.asarray(word_embedding, dtype=np.float32)).astype(
        bf16
    )
    pos = np.asarray(position_embedding, dtype=np.float32)
    typ = np.asarray(token_type_embedding, dtype=np.float32)
    diff = typ[1] - typ[0]  # [H]
    posA = pos + typ[0][None, :]  # [SEQ, H]

    # token (core c, b, s_local) -> tile t = b//2, partition p = (b%2)*64 + s_local
    ids_np = np.asarray(input_ids, dtype=np.int32).reshape(
        BATCH, N_CORES, S_PER_CORE
    )  # [b, c, s]
    tt_np = np.asarray(token_type_ids, dtype=np.float32).reshape(
        BATCH, N_CORES, S_PER_CORE
    )

    def to_tp(x):  # [b, c, s] -> [c, t, p]
        x = x.reshape(T_TILES, 2, N_CORES, S_PER_CORE)  # [t, par, c, s]
        return x.transpose(2, 0, 1, 3).reshape(N_CORES, T_TILES, P)

    ids_tp = to_tp(ids_np)  # [c, 16, 128]
    tt_tp = to_tp(tt_np)

    ids_dram = np.ascontiguousarray(ids_tp.transpose(0, 2, 1))  # [c, p, t] int32

    # lhsb [66, C*128]: rows 0-63 onehot(p%64); rows 64/65 = tt of the
    # chunk's first/second tile (row 65 zero for 1-tile chunks: its rhs
    # half is outside the chunk's slice anyway)
    onehot = np.zeros((64, P), dtype=np.float32)
    onehot[np.arange(P) % 64, np.arange(P)] = 1.0
    lhsb = np.zeros((N_CORES, K_LHS, C_CHUNKS * P), dtype=bf16)
    for g in range(C_CHUNKS):
        lhsb[:, :64, g * P : (g + 1) * P] = onehot.astype(bf16)[None]
        lhsb[:, 64, g * P : (g + 1) * P] = tt_tp[:, CHUNK_T0[g], :].astype(bf16)
        if CHUNK_TILES[g] > 1:
            lhsb[:, 65, g * P : (g + 1) * P] = tt_tp[:, CHUNK_T0[g] + 1, :].astype(
                bf16
            )

    # rhsb [66, 1536]: rows 0-63 = [posA64 | posA64]; row 64 = [diff|0]; row 65 = [0|diff]
    rhsb = np.zeros((N_CORES, K_LHS, 2 * H), dtype=bf16)
    for c in range(N_CORES):
        pa = posA[c * S_PER_CORE : (c + 1) * S_PER_CORE].astype(bf16)  # [64, H]
        pa2 = np.concatenate([pa, pa], axis=0)  # [128, H]
        rhsb[c, :64, :H] = pa
        rhsb[c, :64, H:] = pa
    rhsb[:, 64, :H] = diff.astype(bf16)[None]
    rhsb[:, 65, H:] = diff.astype(bf16)[None]

    mmb = np.concatenate([lhsb, rhsb], axis=2)  # [c, 66, C*128 + 2H]
    return [
        {
            "wemb": wemb,
            "mmb": np.ascontiguousarray(mmb[c]),
            "ids": np.ascontiguousarray(ids_dram[c]),
        }
        for c in range(N_CORES)
    ]


def kernel(
    input_ids, token_type_ids, word_embedding, position_embedding, token_type_embedding
):
    from concourse.bass_utils import run_bass_kernel_spmd

    nc = _get_nc()
    in_maps = _prep_inputs(
        input_ids,
        token_type_ids,
        word_embedding,
        position_embedding,
        token_type_embedding,
    )
    r = run_bass_kernel_spmd(nc, in_maps, core_ids=list(range(N_CORES)))
    # per-core out [128, 16, 768]: token (t*128+p) = (b=2t+p//64, s=p%64)
    cores = []
    for c in range(N_CORES):
        o = np.asarray(r.results[c]["out"])  # [128, 16, 768]
        o = o.reshape(2, S_PER_CORE, T_TILES, H)  # [par, s, t, h]
        o = o.transpose(2, 0, 1, 3).reshape(BATCH, S_PER_CORE, H)  # [b, s, h]
        cores.append(o)
    full = np.stack(cores, axis=0)  # [c, b, s, h]
    full = full.transpose(1, 0, 2, 3).reshape(BATCH, SEQ, H)
    return np.ascontiguousarray(full)


# revision 6
# speedup vs baseline: 1.1456x; 1.0143x over previous
"""BERT embedding lookup on 8 TRN2 NeuronCores — bf16 + PE-base version.

Strategy: sequence-parallel — core c handles positions [64c, 64c+64) of
all 32 batch rows (2048 tokens, 16 tiles of 128; tile t = batch rows
(2t, 2t+1), partition p = (batch parity p//64, s_local p%64)).

Word table converted to bf16 on the host (rel err ~2^-9, inside the 2e-2
gate): halves gather HBM bytes. Gathers are 16 generic SWDGE indirect
DMAs of 128 rows each (the HW ucode supports exactly one offset per
partition per op), issued back-to-back with no pool-recycling stalls so
the ~1.4us/op Q7 descriptor-generation pace is the only serialization.

The position+type base (posA[s] + tt*diff) is built on the otherwise
IDLE PE as K=66 matmuls per PSUM bank: lhsT rows 0-63 are a onehot(p%64)
pattern selecting posA rows, rows 64-65 are the chunk's two tt columns
selecting diff. DVE then does a single pass per chunk:
res = wt*1 + psum_base (stt) — a second DVE pass would cost ~15us
extra against the roofline. Results are stored in BF16 (the host upcasts
to f32 during reassembly): halves store HBM bytes and SDMA engine time;
the extra bf16 rounding of the sum adds ~2e-3 rel err, still ~7x inside
the 2e-2 gate.

Compute/store chunks taper at both ends (1,1,2,2,2,2,2,2,1,1 tiles)
so stores start early (filling the SDMA engines during the gather ramp)
and the tail after the last gather is short; stores alternate
sync/scalar HWDGE queues.
Host reassembles [32, 512, 768] from the [128, 16, 768] per-core outs.
"""

import numpy as np

P = 128
H = 768
VOCAB = 30522
SEQ = 512
BATCH = 32
N_CORES = 8
S_PER_CORE = SEQ // N_CORES  # 64
TOK_PER_CORE = BATCH * S_PER_CORE  # 2048
T_TILES = TOK_PER_CORE // P  # 16
CHUNK_TILES = [1, 1, 2, 2, 2, 2, 2, 2, 1, 1]  # per compute/store chunk
C_CHUNKS = len(CHUNK_TILES)
CHUNK_T0 = [sum(CHUNK_TILES[:i]) for i in range(C_CHUNKS)]
K_LHS = 66  # 64 onehot rows + 2 tt rows

_CACHE = {}


def _build():
    from concourse import bacc, mybir
    import concourse.bass as bass
    import concourse.tile as tile

    nc = bacc.Bacc(
        "TRN2",
        target_bir_lowering=False,
        debug=False,
        num_devices=N_CORES,
        dynamic_dma_scratch_size=131072,
        num_swdge_queues=1,
    )
    f32 = mybir.dt.float32
    bf16 = mybir.dt.bfloat16
    i32 = mybir.dt.int32

    wemb = nc.dram_tensor("wemb", [VOCAB, H], bf16, kind="ExternalInput").ap()
    # fused matmul consts: lhs [66, C*128] | rhs [66, 1536]
    mmb = nc.dram_tensor(
        "mmb", [K_LHS, C_CHUNKS * P + 2 * H], bf16, kind="ExternalInput"
    ).ap()
    ids = nc.dram_tensor("ids", [P, T_TILES], i32, kind="ExternalInput").ap()
    out = nc.dram_tensor("out", [P, T_TILES, H], bf16, kind="ExternalOutput").ap()

    with tile.TileContext(nc) as tc:
        with (
            tc.tile_pool(name="consts", bufs=1) as consts,
            tc.tile_pool(name="resp", bufs=5) as rpool,
            tc.tile_pool(name="psum", bufs=2, space="PSUM") as ppool,
        ):
            ids_sb = consts.tile([P, T_TILES], i32)
            # scalar HWDGE issues earliest after the prologue; ids first so
            # the gather stream unblocks as soon as possible
            nc.scalar.dma_start(out=ids_sb[:], in_=ids[:])
            mm_sb = consts.tile([K_LHS, C_CHUNKS * P + 2 * H], bf16)
            nc.scalar.dma_start(out=mm_sb[:], in_=mmb[:])
            lhs_sb = mm_sb[:, : C_CHUNKS * P]
            rhs_sb = mm_sb[:, C_CHUNKS * P :]

            # all gather destinations live for the whole kernel: the 16
            # gathers issue back-to-back on Q7 with no pool-recycling stalls
            wts = [
                consts.tile([P, CHUNK_TILES[g] * H], bf16, name=f"wt{g}")
                for g in range(C_CHUNKS)
            ]
            for g in range(C_CHUNKS):
                for k in range(CHUNK_TILES[g]):
                    t = CHUNK_T0[g] + k
                    # HW ucode semantics: ONE offset per partition per op
                    nc.gpsimd.indirect_dma_start(
                        out=wts[g][:, k * H : (k + 1) * H],
                        out_offset=None,
                        in_=wemb[:],
                        in_offset=bass.IndirectOffsetOnAxis(
                            ap=ids_sb[:, t : t + 1], axis=0
                        ),
                    )
            for g in range(C_CHUNKS):
                cw = CHUNK_TILES[g] * H
                base_full = ppool.tile([P, 2 * H], f32, name="base")
                base = base_full[:, :cw]
                for b0 in range(0, cw, 512):
                    b1 = min(b0 + 512, cw)
                    # base[p, n] = posA[p, n] + tt0[p]*diffA[n] + tt1[p]*diffB[n]
                    nc.tensor.matmul(
                        base_full[:, b0:b1],
                        lhs_sb[:, g * P : (g + 1) * P],
                        rhs_sb[:, b0:b1],
                        start=True,
                        stop=True,
                    )
                res_full = rpool.tile([P, 2 * H], bf16, name="res")
                # res = wt * 1 + base  (single DVE pass, f32 out)
                nc.vector.scalar_tensor_tensor(
                    out=res_full[:, :cw],
                    in0=wts[g][:],
                    scalar=1.0,
                    in1=base,
                    op0=mybir.AluOpType.mult,
                    op1=mybir.AluOpType.add,
                )
                eng = nc.sync if g % 2 == 0 else nc.scalar
                t0 = CHUNK_T0[g]
                eng.dma_start(
                    out=out[:, t0 : t0 + CHUNK_TILES[g], :],
                    in_=res_full[:, :cw],
                )

    nc.compile()
    return nc


def _get_nc():
    if "nc" not in _CACHE:
        _CACHE["nc"] = _build()
    return _CACHE["nc"]


def _prep_inputs(
    input_ids, token_type_ids, word_embedding, position_embedding, token_type_embedding
):
    import ml_dtypes

    bf16 = ml_dtypes.bfloat16
    wemb = np.ascontiguousarray(np.asarray(word_embedding, dtype=np.float32)).astype(
        bf16
    )
    pos = np.asarray(position_embedding, dtype=np.float32)
    typ = np.asarray(token_type_embedding, dtype=np.float32)
    diff = typ[1] - typ[0]  # [H]
    posA = pos + typ[0][None, :]  # [SEQ, H]

    # token (core c, b, s_local) -> tile t = b//2, partition p = (b%2)*64 + s_local
    ids_np = np.asarray(input_ids, dtype=np.int32).reshape(
        BATCH, N_CORES, S_PER_CORE
    )  # [b, c, s]
    tt_np = np.asarray(token_type_ids, dtype=np.float32).reshape(
        BATCH, N_CORES, S_PER_CORE
    )

    def to_tp(x):  # [b, c, s] -> [c, t, p]
        x = x.reshape(T_TILES, 2, N_CORES, S_PER_CORE)  # [t, par, c, s]
        return x.transpose(2, 0, 1, 3).reshape(N_CORES, T_TILES, P)

    ids_tp = to_tp(ids_np)  # [c, 16, 128]
    tt_tp = to_tp(tt_np)

    ids_dram = np.ascontiguousarray(ids_tp.transpose(0, 2, 1))  # [c, p, t] int32

    # lhsb [66, C*128]: rows 0-63 onehot(p%64); rows 64/65 = tt of the
    # chunk's first/second tile (row 65 zero for 1-tile chunks: its rhs
    # half is outside the chunk's slice anyway)
    onehot = np.zeros((64, P), dtype=np.float32)
    onehot[np.arange(P) % 64, np.arange(P)] = 1.0
    lhsb = np.zeros((N_CORES, K_LHS, C_CHUNKS * P), dtype=bf16)
    for g in range(C_CHUNKS):
        lhsb[:, :64, g * P : (g + 1) * P] = onehot.astype(bf16)[None]
        lhsb[:, 64, g * P : (g + 1) * P] = tt_tp[:, CHUNK_T0[g], :].astype(bf16)
        if CHUNK_TILES[g] > 1:
            lhsb[:, 65, g * P : (g + 1) * P] = tt_tp[:, CHUNK_T0[g] + 1, :].astype(
                bf16
            )

    # rhsb [66, 1536]: rows 0-63 = [posA64 | posA64]; row 64 = [diff|0]; row 65 = [0|diff]
    rhsb = np.zeros((N_CORES, K_LHS, 2 * H), dtype=bf16)
    for c in range(N_CORES):
        pa = posA[c * S_PER_CORE : (c + 1) * S_PER_CORE].astype(bf16)  # [64, H]
        pa2 = np.concatenate([pa, pa], axis=0)  # [128, H]
        rhsb[c, :64, :H] = pa
        rhsb[c, :64, H:] = pa
    rhsb[:, 64, :H] = diff.astype(bf16)[None]
    rhsb[:, 65, H:] = diff.astype(bf16)[None]

    mmb = np.concatenate([lhsb, rhsb], axis=2)  # [c, 66, C*128 + 2H]
    return [
        {
            "wemb": wemb,
            "mmb": np.ascontiguousarray(mmb[c]),
            "ids": np.ascontiguousarray(ids_dram[c]),
        }
        for c in range(N_CORES)
    ]


def kernel(
    input_ids, token_type_ids, word_embedding, position_embedding, token_type_embedding
):
    from concourse.bass_utils import run_bass_kernel_spmd

    nc = _get_nc()
    in_maps = _prep_inputs(
        input_ids,
        token_type_ids,
        word_embedding,
        position_embedding,
        token_type_embedding,
    )
    r = run_bass_kernel_spmd(nc, in_maps, core_ids=list(range(N_CORES)))
    # per-core out [128, 16, 768]: token (t*128+p) = (b=2t+p//64, s=p%64)
    cores = []
    for c in range(N_CORES):
        o = np.asarray(r.results[c]["out"]).astype(np.float32)  # [128, 16, 768]
        o = o.reshape(2, S_PER_CORE, T_TILES, H)  # [par, s, t, h]
        o = o.transpose(2, 0, 1, 3).reshape(BATCH, S_PER_CORE, H)  # [b, s, h]
        cores.append(o)
    full = np.stack(cores, axis=0)  # [c, b, s, h]
    full = full.transpose(1, 0, 2, 3).reshape(BATCH, SEQ, H)
    return np.ascontiguousarray(full)
